# revision 7
# baseline (speedup 1.0000x reference)
"""Bloom attention (separated QKV) — 8-core TRN2 Bass kernel.

Distribution: tensor-parallel over heads (2 heads/core). Each core:
  1. QKV projections for its 256-row slice of Wq/Wk/Wv via fp8e4
     DoubleRow matmuls (2 k-tiles per pass, ~1.6x bf16 rate): hs and
     weights are host-cast to fp8 (weights pre-scaled x64 so e4m3
     normals cover them; undone in the PSUM->SBUF activation). q^T/k^T
     land in [d,s] bf16, v in [s,d] bf16.
  2. Attention with transposed scores St[k,q] = k @ q^T (bf16, K=128 so
     no DoubleRow), exp via ScalarE (alibi as per-partition bias),
     denominator via DVE pair-sums + one [128,128] ones(1/32)-matmul per
     half (reduce + broadcast in one shot), ctx^T = v^T @ P in PSUM,
     normalized by reciprocal and cast to fp8 (x32 so e4m3 normals cover
     ctx).
  3. One AllGather per 1024-query block (both heads, fp8 payload);
     block 3 gathers per head so AG(3,0)'s mesh runs under A(3,1)'s
     compute and only AG(3,1) remains on the critical tail.
  4. Dense projection entirely after attention, ordered d0, d1,
     d3(hi0 matmuls), d2, d3(hi1) so the d3 accumulation needing
     AG(3,1) sits as late as possible while the PE stays busy.
     Residual (with bd folded in, bf16) is preloaded to SBUF during
     attention; closures run on DVE (scalar_tensor_tensor) and outputs
     store as bf16 via the scalar queue (which carries no collectives,
     so output-slot reuse never entangles with AG completion counts).
  5. ctile prefetches all ride the sync queue: pf0..pf2 ungated at
     dense start, pf3h0/pf3h1 gated on their AGs with nothing queued
     behind them.
Startup DMAs are spread over the sync/scalar/gpsimd queues in
need-order (per-queue DMA bandwidth is limited; a gated DMA trigger
blocks its queue head-of-line, so bulk transfers stay off the ACT
queue once activations start).
Host side: transpose/slice/cast weights + hs (layout prep only), fold
bd into the residual, concatenate the 8 output column-slices.
"""
import numpy as np
import ml_dtypes

import concourse.bass as bass
import concourse.bacc as bacc
import concourse.mybir as mybir
import concourse.tile as tile
import concourse.bass_utils as bass_utils

BF16 = ml_dtypes.bfloat16
F8NP = ml_dtypes.float8_e4m3
N_CORES = 8
B, S, H = 2, 2048, 2048
NH, HD = 16, 128
HPC = NH // N_CORES          # heads per core
CI = HPC * HD                # per-core slice of H (256)
BS = B * S                   # 4096
INV_NORM = 1.0 / float(np.sqrt(HD))
WS = 64.0                    # fp8 weight pre-scale (host), undone in ACT
CS = 32.0                    # ctx pre-scale before fp8 cast, undone in ACT

JT = H // 128                # 16 contraction tiles for projections
JP = JT // 2                 # 8 DoubleRow k-tile pairs
SS_CHUNK = 512               # seq chunk for projections
N_CHUNKS = BS // SS_CHUNK    # 8
KT = S // 128                # 16 key tiles per batch
IT = H // 128                # 16 contraction tiles for dense
QBLK = 1024                  # attention/AG/dense block along seq
N_BLOCKS = BS // QBLK        # 4
NSC = QBLK // SS_CHUNK       # 2 seq chunks per block

F32 = mybir.dt.float32
BF = mybir.dt.bfloat16
FP8 = mybir.dt.float8e4
DR = mybir.MatmulPerfMode.DoubleRow
MUL = mybir.AluOpType.mult
ADD = mybir.AluOpType.add


def _build():
    nc = bacc.Bacc("TRN2", target_bir_lowering=False, debug=False,
                   num_devices=N_CORES)

    # hsT/weights are host-packed to the exact SBUF layouts so every DMA
    # is contiguous per partition (strided weight loads measured ~5x slower)
    hsT = nc.dram_tensor("hsT", [128, N_CHUNKS, JT, SS_CHUNK], FP8,
                         kind="ExternalInput").ap()
    wqT = nc.dram_tensor("wqT", [128, JT, CI], FP8, kind="ExternalInput").ap()
    wkT = nc.dram_tensor("wkT", [128, JT, CI], FP8, kind="ExternalInput").ap()
    wvT = nc.dram_tensor("wvT", [128, JT, CI], FP8, kind="ExternalInput").ap()
    wdT = nc.dram_tensor("wdT", [128, IT, CI], FP8, kind="ExternalInput").ap()
    bq = nc.dram_tensor("bq", [CI, 1], F32, kind="ExternalInput").ap()
    bk = nc.dram_tensor("bk", [CI, 1], F32, kind="ExternalInput").ap()
    bv = nc.dram_tensor("bv", [1, CI], BF, kind="ExternalInput").ap()
    alibi = nc.dram_tensor("alibi", [B * HPC, S], F32, kind="ExternalInput").ap()
    # residual with bd pre-added (host), bf16, [d%128, ct, seq]
    residT = nc.dram_tensor("residT", [128, HPC, BS], BF,
                            kind="ExternalInput").ap()
    outT = nc.dram_tensor("outT", [CI, BS], BF, kind="ExternalOutput").ap()

    bounce = nc.dram_tensor("bounce", [N_BLOCKS, HPC, 128, QBLK], FP8,
                            kind="Internal").ap()
    # per-block AllGather output (both heads per core in one collective
    # — AG cost is latency+skew dominated and the meshes serialize on
    # the CC core, so fewer collectives win)
    gath = nc.dram_tensor("gath", [N_BLOCKS, N_CORES, HPC, 128, QBLK], FP8,
                          kind="Internal", addr_space="Shared").ap()

    with tile.TileContext(nc) as tc:
        with (
            tc.tile_pool(name="const", bufs=1) as constp,
            tc.tile_pool(name="qkv", bufs=1) as qkvp,
            tc.tile_pool(name="ctile", bufs=4) as ctp,
        ):
            # ---- phase 0: earliest constants ----
            wq_sb = constp.tile([128, JT, CI], FP8)
            wk_sb = constp.tile([128, JT, CI], FP8)
            wv_sb = constp.tile([128, JT, CI], FP8)
            wd_sb = constp.tile([128, IT, CI], FP8)
            bq_sb = constp.tile([128, HPC], F32)
            bk_sb = constp.tile([128, HPC], F32)
            bv_sb = constp.tile([1, CI], BF)
            alibi_sb = constp.tile([128, B * HPC, KT], F32)
            resid_sb = constp.tile([128, HPC, BS], BF)
            # startup-critical DMAs spread over all three queues in
            # need-order (first q matmul needs wq+hs0; k at ~+5us needs
            # wk; v at ~+9us needs wv+bv; ACT biases have pool slack):
            #   gpsimd: wq, wv, hs1_h1, hs2, hs4, hs6, wd, resid_h1
            #   sync:   hs0_h1, wk_h2, bq, bk, bv, hs1_h2, hs3, ...
            #   scalar: hs0_h2, wk_h1 — then the queue is clear before
            #   the projection ACTIVATEs start
            for h in range(2):
                sl = slice(h * JT // 2, (h + 1) * JT // 2)
                nc.gpsimd.dma_start(wq_sb[:, sl], wqT[:, sl])
            nc.gpsimd.dma_start(wv_sb[:], wvT[:])
            # den reduce+broadcast lhsT: [128,128] of 1/CS — one matmul
            # per half gives denb[m,q] = den[q]/CS on every partition
            ones128 = constp.tile([128, 128], BF)
            ones_row_bf = constp.tile([1, 128], BF)    # v-bias lhsT
            nc.vector.memset(ones128[:], 1.0 / CS)
            nc.vector.memset(ones_row_bf[:], 1.0)

            # persistent per-core activations
            qT_sb = qkvp.tile([128, HPC, BS], BF)      # [d, hi, ss]
            kT_sb = qkvp.tile([128, HPC, BS], BF)
            v_sb = qkvp.tile([128, BS // 128, CI], BF)  # [ss%128, ss//128, i]

            # ---- phase 1: QKV projections ----
            with (
                tc.tile_pool(name="hsb", bufs=4) as hsp,
                tc.tile_pool(name="p1psum", bufs=4,
                             space=bass.MemorySpace.PSUM) as p1p,
            ):
                for ch in range(N_CHUNKS):
                    s0 = ch * SS_CHUNK
                    hsb = hsp.tile([128, JT, SS_CHUNK], FP8, name="hsb")
                    if ch == 0:
                        # critical prefix hs0+wq (1.5MB) split evenly
                        # over the 3 DMA-capable queues: first matmul
                        # needs all of hs0 and wq
                        nc.sync.dma_start(hsb[:, :JT // 2],
                                          hsT[:, 0, :JT // 2])
                        nc.scalar.dma_start(hsb[:, JT // 2:],
                                            hsT[:, 0, JT // 2:])
                        nc.scalar.dma_start(wk_sb[:, :JT // 2],
                                            wkT[:, :JT // 2])
                        nc.sync.dma_start(wk_sb[:, JT // 2:],
                                          wkT[:, JT // 2:])
                        for b_sb, b_dr in ((bq_sb, bq), (bk_sb, bk)):
                            for hi in range(HPC):
                                nc.sync.dma_start(
                                    b_sb[:, hi:hi + 1],
                                    b_dr[hi * 128:(hi + 1) * 128, :])
                        nc.sync.dma_start(bv_sb[:], bv[:])
                    elif ch == 1:
                        nc.gpsimd.dma_start(hsb[:, :JT // 2],
                                            hsT[:, 1, :JT // 2])
                        nc.sync.dma_start(hsb[:, JT // 2:],
                                          hsT[:, 1, JT // 2:])
                    else:
                        # even chunks on gpsimd, odd on sync
                        q = nc.gpsimd if ch % 2 == 0 else nc.sync
                        q.dma_start(hsb[:], hsT[:, ch])
                    for w_sb, b_col, o_sb, scale in (
                        (wq_sb, bq_sb, qT_sb, INV_NORM / WS),
                        (wk_sb, bk_sb, kT_sb, 1.0 / WS),
                    ):
                        for hi in range(HPC):
                            ps = p1p.tile([128, SS_CHUNK], F32, name="ps_qk")
                            for jp in range(JP):
                                nc.tensor.matmul(
                                    ps[:],
                                    w_sb[:, 2 * jp:2 * jp + 2,
                                         hi * 128:(hi + 1) * 128],
                                    hsb[:, 2 * jp:2 * jp + 2, :],
                                    start=(jp == 0), stop=(jp == JP - 1),
                                    perf_mode=DR)
                            nc.scalar.activation(
                                o_sb[:, hi, s0:s0 + SS_CHUNK], ps[:],
                                mybir.ActivationFunctionType.Identity,
                                bias=b_col[:, hi:hi + 1], scale=scale)
                    for st in range(SS_CHUNK // 128):
                        ps = p1p.tile([128, CI], F32, name="ps_v")
                        # bv host-scaled by WS so the 1/WS below restores it
                        nc.tensor.matmul(ps[:], ones_row_bf[:], bv_sb[:],
                                         start=True, stop=False)
                        for jp in range(JP):
                            nc.tensor.matmul(
                                ps[:],
                                hsb[:, 2 * jp:2 * jp + 2,
                                    st * 128:(st + 1) * 128],
                                wv_sb[:, 2 * jp:2 * jp + 2, :],
                                start=False, stop=(jp == JP - 1),
                                perf_mode=DR)
                        nc.scalar.activation(
                            v_sb[:, ch * 4 + st, :], ps[:],
                            mybir.ActivationFunctionType.Identity,
                            scale=1.0 / WS)

            # late consts (attention/dense phases) — emitted after phase
            # 1 so their DMAs don't delay the first projections
            nc.gpsimd.dma_start(wd_sb[:], wdT[:])
            nc.sync.dma_start(
                alibi_sb[:], alibi.rearrange("r (kt p) -> p r kt", p=128))
            nc.sync.dma_start(resid_sb[:, 0], residT[:, 0])
            nc.gpsimd.dma_start(resid_sb[:, 1], residT[:, 1])

            # ---- phase 2+3: attention + per-block AllGather, then the
            # dense output projection ordered so only d3's second half
            # waits on the last AG
            with (
                tc.tile_pool(name="stp", bufs=3,
                             space=bass.MemorySpace.PSUM) as stp,
                tc.tile_pool(name="ptp", bufs=12) as ptp,
                tc.tile_pool(name="accp", bufs=1,
                             space=bass.MemorySpace.PSUM) as accp,
                tc.tile_pool(name="normp", bufs=2) as normp,
                tc.tile_pool(name="outp", bufs=8) as outp,
            ):
                LAG = 6
                pending_tail = [None]
                ctiles = {}

                def flush_tail():
                    if pending_tail[0] is not None:
                        pending_tail[0]()
                        pending_tail[0] = None

                def attn_group(blk, hi):
                    b, qh = divmod(blk, N_BLOCKS // B)
                    q0 = b * S + qh * QBLK
                    bh = b * HPC + hi
                    ctx_ps = accp.tile([128, QBLK], F32, name="ctx_ps")
                    acc_sb = normp.tile([128, QBLK], BF, name="acc_sb")
                    pts = []

                    def consume(kt):
                        pt = pts[kt]
                        for half in range(2):
                            hs_ = slice(half * SS_CHUNK,
                                        (half + 1) * SS_CHUNK)
                            nc.tensor.matmul(
                                ctx_ps[:, hs_],
                                v_sb[:, (b * S) // 128 + kt,
                                     hi * 128:(hi + 1) * 128],
                                pt[:, half, :],
                                start=(kt == 0), stop=(kt == KT - 1))
                        # denominator partial sums on DVE (off PE):
                        # bf16 pair-sum (2x DVE rate)
                        if kt % 2 == 1:
                            pa = pts[kt - 1][:].rearrange("p a b -> p (a b)")
                            pb = pt[:].rearrange("p a b -> p (a b)")
                            psum2 = normp.tile([128, QBLK], BF,
                                               name="psum2")
                            nc.vector.tensor_add(psum2[:], pa, pb)
                            if kt == 1:
                                nc.vector.tensor_copy(acc_sb[:], psum2[:])
                            else:
                                nc.vector.tensor_add(acc_sb[:], acc_sb[:],
                                                     psum2[:])

                    for kt in range(KT):
                        k0 = b * S + kt * 128
                        st_ps = stp.tile([128, 2, SS_CHUNK], F32,
                                         name="st_ps")
                        for half in range(2):
                            nc.tensor.matmul(
                                st_ps[:, half, :],
                                kT_sb[:, hi, k0:k0 + 128],
                                qT_sb[:, hi,
                                      q0 + half * SS_CHUNK:
                                      q0 + (half + 1) * SS_CHUNK],
                                start=True, stop=True)
                        pt = ptp.tile([128, 2, SS_CHUNK], BF, name="pt")
                        # q pre-scaled by INV_NORM in phase 1; alibi is
                        # a per-partition (key-position) bias
                        nc.scalar.activation(
                            pt[:], st_ps[:],
                            mybir.ActivationFunctionType.Exp,
                            bias=alibi_sb[:, bh, kt:kt + 1])
                        pts.append(pt)
                        # previous group's normalize tail slots in
                        # behind our first few St/exp emissions
                        if kt == 2:
                            flush_tail()
                        if kt >= LAG:
                            consume(kt - LAG)
                    for kt in range(KT - LAG, KT):
                        consume(kt)
                    # den reduce+broadcast in one matmul per half:
                    # denb[m,q] = den[q]/CS for all m (ones128 = 1/CS)
                    den_ps = stp.tile([128, 2, SS_CHUNK], F32,
                                      name="st_ps")
                    for half in range(2):
                        nc.tensor.matmul(
                            den_ps[:, half, :], ones128[:],
                            acc_sb[:, half * SS_CHUNK:
                                   (half + 1) * SS_CHUNK],
                            start=True, stop=True)

                    def tail():
                        denb_sb = normp.tile([128, QBLK], F32,
                                             name="denb_sb")
                        nc.vector.reciprocal_approx_fast(
                            denb_sb[:],
                            den_ps[:].rearrange("p a b -> p (a b)"))
                        ctxn_sb = normp.tile([128, QBLK], FP8,
                                             name="ctxn_sb")
                        nc.vector.tensor_mul(ctxn_sb[:], ctx_ps[:],
                                             denb_sb[:])
                        # bounce rides gpsimd (ahead of the AG triggers
                        # there) so gated ctile prefetches on sync can
                        # never head-of-line-block it
                        nc.gpsimd.dma_start(bounce[blk, hi], ctxn_sb[:])
                        if hi == 1:
                            # one AG per block, both heads' bounce slices
                            nc.gpsimd.collective_compute(
                                "AllGather", mybir.AluOpType.bypass,
                                replica_groups=[list(range(N_CORES))],
                                ins=[bounce[blk]],
                                outs=[gath[blk]])
                        if (blk, hi) == (3, 0):
                            # emitted after AG(2)'s trigger so their
                            # conservative collective-completion gate is
                            # AG(2), which lands well before dense starts
                            for pblk in range(3):
                                prefetch_ctile(pblk)

                    pending_tail[0] = tail

                def prefetch_ctile(blk):
                    # DoubleRow pairs: dim1 j = pair of global heads
                    # (2j, 2j+1). All prefetches ride the sync queue
                    # (no collectives there, so completion counts stay
                    # clean; only the gated pf3 ever parks at its head,
                    # with nothing queued behind it).
                    ctile = ctp.tile([128, IT // 2, 2, QBLK], FP8,
                                     name="ctile")
                    srcg = gath[blk].rearrange("c h d q -> d c h q")
                    for p4 in range(4):
                        nc.sync.dma_start(
                            ctile[:, 2 * p4:2 * p4 + 2],
                            srcg[:, 2 * p4:2 * p4 + 2])
                    ctiles[blk] = ctile

                def dense_closure_ct(ct, dpair, q0):
                    # out = dpair/(WS*CS) + resid' on DVE; store bf16 on
                    # the scalar queue (idle after the last exp, and it
                    # carries no collectives so outp slot reuse never
                    # waits on AG completion counting)
                    for sc in range(NSC):
                        c0 = ct * 128
                        s0_ = q0 + sc * SS_CHUNK
                        osb = outp.tile([128, SS_CHUNK], BF,
                                        name="osb")
                        nc.vector.scalar_tensor_tensor(
                            osb[:], dpair[:, sc, :],
                            1.0 / (WS * CS),
                            resid_sb[:, ct, s0_:s0_ + SS_CHUNK],
                            MUL, ADD)
                        nc.scalar.dma_start(
                            outT[c0:c0 + 128, s0_:s0_ + SS_CHUNK],
                            osb[:])

                def dense_block(blk):
                    b, qh = divmod(blk, N_BLOCKS // B)
                    q0 = b * S + qh * QBLK
                    for ct in range(HPC):
                        dpair = stp.tile([128, NSC, SS_CHUNK], F32,
                                         name="st_ps")
                        for j in range(IT // 2):
                            wsl = wd_sb[:, 2 * j:2 * j + 2,
                                        ct * 128:(ct + 1) * 128]
                            for sc in range(NSC):
                                nc.tensor.matmul(
                                    dpair[:, sc, :], wsl,
                                    ctiles[blk][:, j, :, sc * SS_CHUNK:
                                                (sc + 1) * SS_CHUNK],
                                    start=(j == 0), stop=(j == IT // 2 - 1),
                                    perf_mode=DR)
                        dense_closure_ct(ct, dpair, q0)

                # attention: all 8 groups back to back (pf0..pf2 are
                # emitted inside the (3,0) tail, after AG(2)'s trigger)
                for blk in range(N_BLOCKS):
                    for hi in range(HPC):
                        attn_group(blk, hi)
                flush_tail()

                # dense: d0..d2 are ungated by now and cover AG(3)'s
                # mesh; only d3 waits for it
                prefetch_ctile(3)
                for blk in range(N_BLOCKS):
                    dense_block(blk)

    nc.compile()
    return nc


_NC = None


def _get_nc():
    global _NC
    if _NC is None:
        _NC = _build()
    return _NC


def _pack_w(W, sl):
    # [H, CI] transposed slice -> SBUF layout [128, JT, CI], contiguous.
    # Pre-scaled by WS so fp8 e4m3 normals cover the ~1/sqrt(H) magnitudes.
    wT = np.asarray(W, np.float32)[sl].T * WS       # [H, CI]
    return np.ascontiguousarray(
        wT.reshape(JT, 128, CI).transpose(1, 0, 2)).astype(F8NP)


def _prep_in_maps(hidden_states, residual, alibi, Wq, bq, Wk, bk, Wv, bv,
                  Wd, bd):
    hs = np.ascontiguousarray(np.asarray(hidden_states, np.float32)
                              .reshape(BS, H))
    # SBUF chunk layout [128, ch, jt, s]: element = hs[ch*512+s, jt*128+p]
    hs_pack = np.ascontiguousarray(
        hs.reshape(N_CHUNKS, SS_CHUNK, JT, 128).transpose(3, 0, 2, 1)
    ).astype(F8NP)
    resid = np.asarray(residual, np.float32).reshape(BS, H)
    bd_f = np.asarray(bd, np.float32)
    alibi_r = np.asarray(alibi, np.float32).reshape(B, NH, S)
    in_maps = []
    for c in range(N_CORES):
        sl = slice(c * CI, (c + 1) * CI)
        # alibi rows ordered (b, hi) to match kernel indexing bh = b*HPC+hi
        al = np.ascontiguousarray(
            alibi_r[:, c * HPC:(c + 1) * HPC, :].reshape(B * HPC, S))
        # residual slice with bd folded in: [128, ct, seq] bf16
        rb = resid[:, sl] + bd_f[sl][None, :]
        rT = np.ascontiguousarray(
            rb.reshape(BS, HPC, 128).transpose(2, 1, 0)).astype(BF16)
        in_maps.append({
            "hsT": hs_pack,
            "wqT": _pack_w(Wq, sl),
            "wkT": _pack_w(Wk, sl),
            "wvT": _pack_w(Wv, sl),
            "wdT": _pack_w(Wd, sl),
            "bq": np.asarray(bq, np.float32)[sl].reshape(CI, 1),
            "bk": np.asarray(bk, np.float32)[sl].reshape(CI, 1),
            "bv": (np.asarray(bv, np.float32)[sl] * WS).reshape(1, CI)
                  .astype(BF16),
            "alibi": al,
            "residT": rT,
        })
    return in_maps


def run(trace=False, trace_cores=None, stitch_traces=False, **inputs):
    nc = _get_nc()
    in_maps = _prep_in_maps(**inputs)
    res = bass_utils.run_bass_kernel_spmd(
        nc, in_maps, core_ids=list(range(N_CORES)), trace=trace,
        trace_cores=trace_cores, stitch_traces=stitch_traces)
    full = np.empty((BS, H), np.float32)
    for c in range(N_CORES):
        full[:, c * CI:(c + 1) * CI] = (
            res.results[c]["outT"].astype(np.float32).T)
    return full.reshape(B, S, H), res


def kernel(**inputs):
    out, _ = run(trace=False, **inputs)
    return out
